# revision 7
# baseline (speedup 1.0000x reference)
"""DeepseekV2 MLA (non-absorbed prefill form, chunked-softmax MQA) on 8 trn2
NeuronCores.

Sharding: tensor-parallel over heads (16 heads / 8 cores = 2 heads per core);
the 576-wide latent KV cache is replicated per core. Each core computes its two
heads' attention output transposed [256, 1024]; the host concatenates along
heads and transposes back. All matmuls run in bf16 with fp32 PSUM accumulation.

With T=1024 queries (prefill), materializing per-head K/V from the latent
cache is far cheaper than the weight-absorbed decode form: the score
contraction drops 576->192 and PV drops 512->128, at the cost of two
S x 512 x 128 projections per head, amortized over all queries.

Per-core dataflow (transposed [d, t] layouts; no on-chip transposes):
  preprocessing, per s-block (PE + DVE):
    k_nopeT = w_kcT_chunk.T @ kvT_chunk   (PE, accum 4 l-chunks) [128n, 512s]
    v       = kvT_chunk.T @ w_vc_2heads   (PE, accum 4 l-chunks) [128s, 256v]
  main loop, per (head, t-block) phase, per s-tile (PE + ACT + DVE):
    scoresT = ropeT.T @ q_peT  (K=64, paired row groups)
            + k_nopeT.T @ q_nopeT         (PE)                   [128s, 512t]
    pT      = exp(scale * scoresT)        (ACT, PSUM->SBUF bf16)
    acc    += pT                          (DVE, denominator partials)
    attnT  += v_tile.T @ pT               (PE, accum 64 s-tiles) [128v, 512t]
  phase epilogue:
    denom   = ones.T @ acc                (PE, K=128)            [1, 512t]
    recip   = 1/denom                     (DVE)
    bcast   = ones_row.T @ recip          (PE, K=1)              [128, 512t]
    outT    = attnT * bcast               (DVE), DMA out [128v, 512t].
"""

import os
import sys

import numpy as np
import ml_dtypes

for _p in ("/opt/trn_rl_repo",):
    if os.path.isdir(_p) and _p not in sys.path:
        sys.path.append(_p)

import concourse.bass as bass
import concourse.mybir as mybir
import concourse.tile as tile
from concourse.bass_utils import run_bass_kernel_spmd
from concourse.vector_clock import ScopedClock, VectorClock

# ---------------------------------------------------------------- constants
NOPE, ROPE, LORA, VDIM = 128, 64, 512, 128
T, H, S = 1024, 16, 8192
D = LORA + ROPE            # 576 latent dim
SCALING = (NOPE + ROPE) ** -0.5
N_CORES = 8
HPC = H // N_CORES         # heads per core
NST = S // 128             # 64 s-tiles
NSB = S // 512             # 16 s-blocks
NTB = T // 512             # 2 t-blocks
BF16 = mybir.dt.bfloat16
FP32 = mybir.dt.float32
NPBF = ml_dtypes.bfloat16


# ------------------------------------------------- walrus drain workaround
def _patch_tile_drain():
    """The neuronxcc walrus in this container rejects DRAIN instructions
    carrying more than ~2 sync waits ("Too many sync wait commands").
    Split the TileContext exit drain into one drain per processor tick;
    the waits execute sequentially on SP before the all-engine barrier,
    preserving the original semantics."""
    if getattr(tile.TileContext, "_drain_split_patched", False):
        return

    def _drain_and_barrier_split(self, tick_clock, wait_clock):
        gcv = tick_clock.global_clock
        n = len(gcv)
        for proc in range(n):
            t = gcv[proc]
            if t <= 0:
                continue
            vc = VectorClock([0] * n)
            vc.require_at_least(proc, t)
            d = self.nc.sync.drain()
            wait_clock.add_sem_waits(d.ins, ScopedClock({None: vc}))
        self.nc.all_engine_barrier()
        assert self.sems is not None
        popped = self.nc._tile_sem_poison_stack.pop()
        assert popped is self._sem_poison
        self.nc.clear_and_free_semaphores(list(self.sems.allocated().values()))
        self.nc.all_engine_barrier()

    tile.TileContext._drain_and_barrier = _drain_and_barrier_split

    # Same walrus limitation for regular instructions: peel all but the last
    # sync wait off onto same-engine NOPs inserted immediately before the
    # instruction. The engine executes its queue in order, so waiting on the
    # NOPs first is equivalent to one multi-wait instruction.
    orig_add = tile.TileContext._add_instruction

    def _add_instruction_split_waits(self, inst):
        si = inst.sync_info
        if si is not None:
            waits = si.on_wait
            if waits and len(waits) > 1:
                for w in waits[:-1]:
                    nop = mybir.InstNoOp(
                        name=self.nc.get_next_instruction_name(), ins=[], outs=[]
                    )
                    nop.engine = inst.engine
                    nop.sync_info = mybir.SyncInfo(on_wait=[w], on_update=[])
                    orig_add(self, nop)
                inst.sync_info = mybir.SyncInfo(
                    on_wait=[waits[-1]], on_update=si.on_update
                )
        orig_add(self, inst)

    tile.TileContext._add_instruction = _add_instruction_split_waits
    tile.TileContext._drain_split_patched = True


# ------------------------------------------------------------ bass program
MM_KINDS = {}


def _build_program():
    _patch_tile_drain()
    nc = bass.Bass()
    _orig_mm = nc.tensor.matmul

    def _mm_logged(out, lhsT, rhs, kind="?", **kw):
        inst = _orig_mm(out, lhsT, rhs, **kw)
        MM_KINDS[inst.ins.name] = kind
        return inst

    nc.tensor.matmul = _mm_logged
    qnT = nc.declare_dram_parameter("qnT", [HPC, NOPE, T], BF16, isOutput=False)
    qpT = nc.declare_dram_parameter("qpT", [HPC, ROPE, T], BF16, isOutput=False)
    kvT = nc.declare_dram_parameter("kvT", [LORA, S], BF16, isOutput=False)
    kvr = nc.declare_dram_parameter("kvr", [128, S], BF16, isOutput=False)
    wkcT = nc.declare_dram_parameter("wkcT", [HPC, LORA, NOPE], BF16, isOutput=False)
    wvc2 = nc.declare_dram_parameter("wvc2", [4, 128, HPC * VDIM], BF16, isOutput=False)
    out = nc.declare_dram_parameter("out", [HPC * VDIM, T], FP32, isOutput=True)

    Exp = mybir.ActivationFunctionType.Exp


    with tile.TileContext(nc) as tc:
        with (
            tc.tile_pool(name="res", bufs=1) as res,
            tc.tile_pool(name="ptp", bufs=6) as ptp,
            tc.tile_pool(name="gsp", bufs=3) as gsp,
            tc.tile_pool(name="smsb", bufs=4) as smsb,
            tc.tile_pool(name="outsb", bufs=2) as outsb,
            tc.tile_pool(name="ps_sc", bufs=3, space="PSUM") as ps_sc,
            tc.tile_pool(name="ps_attn", bufs=2, space="PSUM") as ps_attn,
        ):
            # ---------------- resident loads. wkcT + kv block 0 first: they
            # unblock the preprocessing matmuls; q is only needed ~60us in.
            wkcT_sb = res.tile([128, HPC, 4, NOPE], BF16, tag="wkct")
            for h in range(HPC):
                for c in range(4):
                    nc.sync.dma_start(
                        wkcT_sb[:, h, c, :], wkcT[h, c * 128:(c + 1) * 128, :]
                    )

            kvT_sb = [
                res.tile([128, S], BF16, tag=f"kvt{c}", name=f"kvt{c}")
                for c in range(4)
            ]
            kvr_sb = res.tile([128, S], BF16, tag="kvr")

            def load_kv_block(b):
                nc.sync.dma_start(
                    kvr_sb[:, b * 1024:(b + 1) * 1024],
                    kvr[:, b * 1024:(b + 1) * 1024],
                )
                for c in range(4):
                    nc.sync.dma_start(
                        kvT_sb[c][:, b * 1024:(b + 1) * 1024],
                        kvT[c * 128:(c + 1) * 128, b * 1024:(b + 1) * 1024],
                    )

            load_kv_block(0)
            wvc_sb = res.tile([128, 4, HPC * VDIM], BF16, tag="wvc")
            for c in range(4):
                nc.sync.dma_start(wvc_sb[:, c, :], wvc2[c])
            for b in range(1, 4):
                load_kv_block(b)
            qnT_sb = res.tile([NOPE, HPC * T], BF16, tag="qnt")
            for h in range(HPC):
                nc.sync.dma_start(qnT_sb[:, h * T:(h + 1) * T], qnT[h])
            # rope rows are duplicated to partitions 64:128 so pairs of K=64
            # rope matmuls can run concurrently in disjoint PE row-groups
            qpe_sb = []
            for h in range(HPC):
                qp = res.tile([128, T], BF16, tag=f"qpe{h}")
                nc.scalar.dma_start(qp[0:ROPE, :], qpT[h])
                nc.scalar.dma_start(qp[ROPE:128, :], qpT[h])
                qpe_sb.append(qp)
            for b in range(4, 8):
                load_kv_block(b)
            ones_col = res.tile([128, 1], FP32, tag="ones_col")
            nc.vector.memset(ones_col[:], 1.0)
            ones_row = res.tile([1, 128], FP32, tag="ones_row")
            nc.vector.memset(ones_row[:], 1.0)

            # PE warmup: ~6us of matmuls on local data so HAM un-throttles and
            # the first real matmuls run at 2.4GHz instead of 1.2; also covers
            # the initial DMA latency of wkcT + kv block 0
            warm = res.tile([128, 512], BF16, tag="warm")
            nc.vector.memset(warm[:], 0.0)
            wu_ps = ps_sc.tile([128, 512], FP32, tag="sc", name="wu_ps")
            for _w in range(18):
                nc.tensor.matmul(wu_ps[:], warm[:, 0:128], warm[:, 0:512],
                                 kind="warm")

            # ---------------- preprocessing: per-head K_nope^T and V
            # k_nopeT[h] = w_kc[h] @ kv_lora^T   [128n, S]
            # v[s, :] = kv_lora[s] @ w_vc[h]     [128s, HPC*128v] per s-tile
            knT_sb = [
                res.tile([NOPE, S], BF16, tag=f"knt{h}", name=f"knt{h}")
                for h in range(HPC)
            ]
            v_sb = res.tile([128, NST, HPC * VDIM], BF16, tag="vsb")
            for sb in range(NSB):
                lo, hi = sb * 512, (sb + 1) * 512
                for h in range(HPC):
                    kn_ps = ps_sc.tile([128, 512], FP32, tag="sc", name="kn_ps")
                    for c in range(4):
                        nc.tensor.matmul(
                            kn_ps[:],
                            wkcT_sb[:, h, c, :],
                            kvT_sb[c][:, lo:hi],
                            kind="knope",
                            start=(c == 0),
                            stop=(c == 3),
                        )
                    nc.vector.tensor_copy(knT_sb[h][:, lo:hi], kn_ps[:])
                for k in range(4):
                    st = sb * 4 + k
                    v_ps = ps_sc.tile([128, HPC * VDIM], FP32, tag="sc",
                                      name="v_ps")
                    for c in range(4):
                        nc.tensor.matmul(
                            v_ps[:],
                            kvT_sb[c][:, st * 128:(st + 1) * 128],
                            wvc_sb[:, c, :],
                            kind="vproj",
                            start=(c == 0),
                            stop=(c == 3),
                        )
                    nc.scalar.copy(v_sb[:, st, :], v_ps[:])

            # ---------------- main phases: (head, t-block)
            # s-tiles are processed in PAIRS: scores for two s-tiles land in
            # one 2-bank PSUM tile so a single ACTIVATE exponentiates both
            # (halves the per-call ACT overhead); the denominator pair-sums
            # run on the otherwise-idle GpSimd so DVE only sees one
            # accumulate per pair.
            for ph in range(HPC * NTB):
                h, tb = divmod(ph, NTB)
                tlo, thi = tb * 512, (tb + 1) * 512
                attn_ps = ps_attn.tile([128, 512], FP32, tag="attn")
                acc = smsb.tile([128, 512], FP32, tag="acc")
                pending = []  # (s0, pt2) of previous pairs: PV runs two
                # pairs behind so its exp dependency is long satisfied

                def emit_pv(s0, pt2):
                    for i in range(2):
                        s = s0 + i
                        nc.tensor.matmul(
                            attn_ps[:],
                            v_sb[:, s, h * VDIM:(h + 1) * VDIM],
                            pt2[:, i, :],
                            kind="pv",
                            start=(s == 0),
                            stop=(s == NST - 1),
                        )

                for pp in range(NST // 2):
                    s0 = 2 * pp
                    sc2 = ps_sc.tile([128, 2, 512], FP32, tag="sc",
                                     name="sc2_ps")
                    # the two K=64 rope matmuls run concurrently in disjoint
                    # PE row-groups (rope rows are duplicated to 64:128)
                    for i in range(2):
                        rlo = i * ROPE
                        nc.tensor.matmul(
                            sc2[:, i, :],
                            kvr_sb[rlo:rlo + ROPE,
                                   (s0 + i) * 128:(s0 + i + 1) * 128],
                            qpe_sb[h][rlo:rlo + ROPE, tlo:thi],
                            kind="rope",
                            start=True,
                            stop=False,
                            tile_position=(rlo, 0),
                        )
                    for i in range(2):
                        s = s0 + i
                        nc.tensor.matmul(
                            sc2[:, i, :],
                            knT_sb[h][:, s * 128:(s + 1) * 128],
                            qnT_sb[:, h * T + tlo:h * T + thi],
                            kind="nope",
                            start=False,
                            stop=True,
                        )
                    pt2 = ptp.tile([128, 2, 512], BF16, tag="pt", name="pt2")
                    nc.scalar.activation(pt2[:], sc2[:], Exp, scale=SCALING)
                    gsum = gsp.tile([128, 512], BF16, tag="gsum")
                    nc.gpsimd.tensor_add(gsum[:], pt2[:, 0, :], pt2[:, 1, :])
                    if pp == 0:
                        nc.vector.tensor_copy(acc[:], gsum[:])
                    else:
                        nc.vector.tensor_add(acc[:], acc[:], gsum[:])

                    pending.append((s0, pt2))
                    if len(pending) > 2:
                        emit_pv(*pending.pop(0))
                for p in pending:
                    emit_pv(*p)

                # phase epilogue: total denominator via K=128 matmul, its
                # reciprocal on ACT (partition-parallel after the K=1
                # broadcast matmul would be too late: recip the [1,512] row
                # on ACT where it costs 512 cycles, then broadcast), scale
                # attnT on DVE, DMA out
                den_ps = ps_sc.tile([1, 512], FP32, tag="sc", name="den_ps")
                nc.tensor.matmul(den_ps[:], ones_col[:], acc[:], kind="den")
                den_sb = smsb.tile([1, 512], FP32, tag="den")
                nc.scalar.copy(den_sb[:], den_ps[:])
                bc_ps = ps_sc.tile([128, 512], FP32, tag="sc", name="bc_ps")
                nc.tensor.matmul(bc_ps[:], ones_row[:], den_sb[:], kind="bcast")
                bc_sb = smsb.tile([128, 512], FP32, tag="bc")
                nc.vector.reciprocal(bc_sb[:], bc_ps[:])
                ot = outsb.tile([128, 512], FP32, tag="out")
                nc.vector.tensor_mul(ot[:], attn_ps[:], bc_sb[:])
                nc.sync.dma_start(
                    out[h * VDIM:(h + 1) * VDIM, tlo:thi], ot[:]
                )
    return nc


_PROGRAM = None


def _get_program():
    global _PROGRAM
    if _PROGRAM is None:
        _PROGRAM = _build_program()
    return _PROGRAM


# ---------------------------------------------------------------- host side
last_results = None  # BassKernelResults of the most recent run (for test.py)


def kernel(q, kv_cache, w_kc, w_vc):
    q = np.asarray(q, dtype=np.float32)
    kv_cache = np.asarray(kv_cache, dtype=np.float32)
    w_kc = np.asarray(w_kc, dtype=np.float32)
    w_vc = np.asarray(w_vc, dtype=np.float32)

    kvT_full = np.ascontiguousarray(kv_cache.T).astype(NPBF)       # [576, S]
    kvT_np = kvT_full[:LORA]                                        # [512, S]
    kvr_np = np.concatenate([kvT_full[LORA:], kvT_full[LORA:]], 0)  # [128, S] rope x2

    in_maps = []
    for core in range(N_CORES):
        hs = [core * HPC + i for i in range(HPC)]
        qnT_np = np.stack(
            [np.ascontiguousarray(q[:, h, :NOPE].T) for h in hs]
        ).astype(NPBF)                                              # [HPC,128,T]
        qpT_np = np.stack(
            [np.ascontiguousarray(q[:, h, NOPE:].T) for h in hs]
        ).astype(NPBF)                                              # [HPC,64,T]
        wkcT_np = np.stack(
            [np.ascontiguousarray(w_kc[h].T) for h in hs]
        ).astype(NPBF)                                              # [HPC,512,128]
        wvc2_np = np.stack(
            [
                np.concatenate(
                    [w_vc[h][c * 128:(c + 1) * 128, :] for h in hs], axis=1
                )
                for c in range(4)
            ]
        ).astype(NPBF)                                              # [4,128,HPC*128]
        in_maps.append(
            {
                "qnT": qnT_np,
                "qpT": qpT_np,
                "kvT": kvT_np,
                "kvr": kvr_np,
                "wkcT": wkcT_np,
                "wvc2": wvc2_np,
            }
        )

    nc = _get_program()
    trace = bool(int(os.environ.get("KERNEL_TRACE", "0")))
    trace_cores = None
    if trace and os.environ.get("KERNEL_TRACE_CORES"):
        trace_cores = [
            int(x) for x in os.environ["KERNEL_TRACE_CORES"].split(",")
        ]
    res = run_bass_kernel_spmd(
        nc,
        in_maps,
        core_ids=list(range(N_CORES)),
        trace=trace,
        trace_cores=trace_cores,
    )
    global last_results
    last_results = res

    # per-core out is [HPC*VDIM, T] transposed; concat heads then transpose
    full = np.concatenate([res.results[c]["out"] for c in range(N_CORES)], axis=0)
    return np.ascontiguousarray(full.T.astype(np.float32))


# revision 11
# speedup vs baseline: 1.1923x; 1.1923x over previous
"""DeepseekV2 MLA (non-absorbed prefill form, chunked-softmax MQA) on 8 trn2
NeuronCores.

Sharding: tensor-parallel over heads (16 heads / 8 cores = 2 heads per core);
the 576-wide latent KV cache is replicated per core. Each core computes its two
heads' attention output transposed [256, 1024]; the host concatenates along
heads and transposes back. All matmuls run in bf16 with fp32 PSUM accumulation.

With T=1024 queries (prefill), materializing per-head K/V from the latent
cache is far cheaper than the weight-absorbed decode form: the score
contraction drops 576->192 and PV drops 512->128, at the cost of two
S x 512 x 128 projections per head, amortized over all queries.

Per-core dataflow (transposed [d, t] layouts; no on-chip transposes):
  preprocessing, per s-block (PE + DVE):
    k_nopeT = w_kcT_chunk.T @ kvT_chunk   (PE, accum 4 l-chunks) [128n, 512s]
    v       = kvT_chunk.T @ w_vc_2heads   (PE, accum 4 l-chunks) [128s, 256v]
  main loop, per (head, t-block) phase, per s-tile (PE + ACT + DVE):
    scoresT = ropeT.T @ q_peT  (K=64, paired row groups)
            + k_nopeT.T @ q_nopeT         (PE)                   [128s, 512t]
    pT      = exp(scale * scoresT)        (ACT, PSUM->SBUF bf16)
    acc    += pT                          (DVE, denominator partials)
    attnT  += v_tile.T @ pT               (PE, accum 64 s-tiles) [128v, 512t]
  phase epilogue:
    denom   = ones.T @ acc                (PE, K=128)            [1, 512t]
    recip   = 1/denom                     (DVE)
    bcast   = ones_row.T @ recip          (PE, K=1)              [128, 512t]
    outT    = attnT * bcast               (DVE), DMA out [128v, 512t].
"""

import os
import sys

import numpy as np
import ml_dtypes

for _p in ("/opt/trn_rl_repo",):
    if os.path.isdir(_p) and _p not in sys.path:
        sys.path.append(_p)

import concourse.bass as bass
import concourse.mybir as mybir
import concourse.tile as tile
from concourse.bass_utils import run_bass_kernel_spmd
from concourse.vector_clock import ScopedClock, VectorClock

# ---------------------------------------------------------------- constants
NOPE, ROPE, LORA, VDIM = 128, 64, 512, 128
T, H, S = 1024, 16, 8192
D = LORA + ROPE            # 576 latent dim
SCALING = (NOPE + ROPE) ** -0.5
N_CORES = 8
HPC = H // N_CORES         # heads per core
NST = S // 128             # 64 s-tiles
NSB = S // 512             # 16 s-blocks
NTB = T // 512             # 2 t-blocks
BF16 = mybir.dt.bfloat16
FP32 = mybir.dt.float32
NPBF = ml_dtypes.bfloat16


# ------------------------------------------------- walrus drain workaround
def _patch_tile_drain():
    """The neuronxcc walrus in this container rejects DRAIN instructions
    carrying more than ~2 sync waits ("Too many sync wait commands").
    Split the TileContext exit drain into one drain per processor tick;
    the waits execute sequentially on SP before the all-engine barrier,
    preserving the original semantics."""
    if getattr(tile.TileContext, "_drain_split_patched", False):
        return

    def _drain_and_barrier_split(self, tick_clock, wait_clock):
        gcv = tick_clock.global_clock
        n = len(gcv)
        for proc in range(n):
            t = gcv[proc]
            if t <= 0:
                continue
            vc = VectorClock([0] * n)
            vc.require_at_least(proc, t)
            d = self.nc.sync.drain()
            wait_clock.add_sem_waits(d.ins, ScopedClock({None: vc}))
        self.nc.all_engine_barrier()
        assert self.sems is not None
        popped = self.nc._tile_sem_poison_stack.pop()
        assert popped is self._sem_poison
        self.nc.clear_and_free_semaphores(list(self.sems.allocated().values()))
        self.nc.all_engine_barrier()

    tile.TileContext._drain_and_barrier = _drain_and_barrier_split

    # Same walrus limitation for regular instructions: peel all but the last
    # sync wait off onto same-engine NOPs inserted immediately before the
    # instruction. The engine executes its queue in order, so waiting on the
    # NOPs first is equivalent to one multi-wait instruction.
    orig_add = tile.TileContext._add_instruction

    def _add_instruction_split_waits(self, inst):
        si = inst.sync_info
        if si is not None:
            waits = si.on_wait
            if waits and len(waits) > 1:
                for w in waits[:-1]:
                    nop = mybir.InstNoOp(
                        name=self.nc.get_next_instruction_name(), ins=[], outs=[]
                    )
                    nop.engine = inst.engine
                    nop.sync_info = mybir.SyncInfo(on_wait=[w], on_update=[])
                    orig_add(self, nop)
                inst.sync_info = mybir.SyncInfo(
                    on_wait=[waits[-1]], on_update=si.on_update
                )
        orig_add(self, inst)

    tile.TileContext._add_instruction = _add_instruction_split_waits
    tile.TileContext._drain_split_patched = True


# ------------------------------------------------------------ bass program
MM_KINDS = {}


def _build_program():
    _patch_tile_drain()
    nc = bass.Bass()
    _orig_mm = nc.tensor.matmul

    def _mm_logged(out, lhsT, rhs, kind="?", **kw):
        inst = _orig_mm(out, lhsT, rhs, **kw)
        MM_KINDS[inst.ins.name] = kind
        return inst

    nc.tensor.matmul = _mm_logged
    qnT = nc.declare_dram_parameter("qnT", [HPC, NOPE, T], BF16, isOutput=False)
    qpT = nc.declare_dram_parameter("qpT", [HPC, ROPE, T], BF16, isOutput=False)
    kvT = nc.declare_dram_parameter("kvT", [LORA, S], BF16, isOutput=False)
    kvr = nc.declare_dram_parameter("kvr", [128, S], BF16, isOutput=False)
    wkcT = nc.declare_dram_parameter("wkcT", [HPC, LORA, NOPE], BF16, isOutput=False)
    wvc2 = nc.declare_dram_parameter("wvc2", [4, 128, HPC * VDIM], BF16, isOutput=False)
    out = nc.declare_dram_parameter("out", [HPC * VDIM, T], FP32, isOutput=True)

    Exp = mybir.ActivationFunctionType.Exp


    with tile.TileContext(nc) as tc:
        with (
            tc.tile_pool(name="res", bufs=1) as res,
            tc.tile_pool(name="ptp", bufs=6) as ptp,
            tc.tile_pool(name="accp", bufs=8) as accp,
            tc.tile_pool(name="smsb", bufs=4) as smsb,
            tc.tile_pool(name="outsb", bufs=2) as outsb,
            tc.tile_pool(name="ps_sc", bufs=3, space="PSUM") as ps_sc,
            tc.tile_pool(name="ps_attn", bufs=2, space="PSUM") as ps_attn,
        ):
            # ---------------- resident loads. wkcT + kv block 0 first: they
            # unblock the preprocessing matmuls; q is only needed ~60us in.
            wkcT_sb = res.tile([128, HPC, 4, NOPE], BF16, tag="wkct")
            for h in range(HPC):
                for c in range(4):
                    nc.sync.dma_start(
                        wkcT_sb[:, h, c, :], wkcT[h, c * 128:(c + 1) * 128, :]
                    )

            kvT_sb = [
                res.tile([128, S], BF16, tag=f"kvt{c}", name=f"kvt{c}")
                for c in range(4)
            ]
            kvr_sb = res.tile([128, S], BF16, tag="kvr")

            def load_kv_block(b):
                nc.sync.dma_start(
                    kvr_sb[:, b * 1024:(b + 1) * 1024],
                    kvr[:, b * 1024:(b + 1) * 1024],
                )
                for c in range(4):
                    nc.sync.dma_start(
                        kvT_sb[c][:, b * 1024:(b + 1) * 1024],
                        kvT[c * 128:(c + 1) * 128, b * 1024:(b + 1) * 1024],
                    )

            load_kv_block(0)
            wvc_sb = res.tile([128, 4, HPC * VDIM], BF16, tag="wvc")
            for c in range(4):
                nc.sync.dma_start(wvc_sb[:, c, :], wvc2[c])
            for b in range(1, 4):
                load_kv_block(b)
            qnT_sb = res.tile([NOPE, HPC * T], BF16, tag="qnt")
            for h in range(HPC):
                nc.sync.dma_start(qnT_sb[:, h * T:(h + 1) * T], qnT[h])
            # rope rows are duplicated to partitions 64:128 so pairs of K=64
            # rope matmuls can run concurrently in disjoint PE row-groups
            qpe_sb = []
            for h in range(HPC):
                qp = res.tile([128, T], BF16, tag=f"qpe{h}")
                nc.scalar.dma_start(qp[0:ROPE, :], qpT[h])
                nc.scalar.dma_start(qp[ROPE:128, :], qpT[h])
                qpe_sb.append(qp)
            for b in range(4, 8):
                load_kv_block(b)
            ones_col = res.tile([128, 1], BF16, tag="ones_col")
            nc.vector.memset(ones_col[:], 1.0)
            ones_row = res.tile([1, 128], FP32, tag="ones_row")
            nc.vector.memset(ones_row[:], 1.0)

            # PE warmup: ~6us of matmuls on local data so HAM un-throttles and
            # the first real matmuls run at 2.4GHz instead of 1.2; also covers
            # the initial DMA latency of wkcT + kv block 0
            warm = res.tile([128, 512], BF16, tag="warm")
            nc.vector.memset(warm[:], 0.0)
            wu_ps = ps_sc.tile([128, 512], FP32, tag="sc", name="wu_ps")
            for _w in range(18):
                nc.tensor.matmul(wu_ps[:], warm[:, 0:128], warm[:, 0:512],
                                 kind="warm")

            # ---------------- preprocessing: per-head K_nope^T and V
            # k_nopeT[h] = w_kc[h] @ kv_lora^T   [128n, S]
            # v[s, :] = kv_lora[s] @ w_vc[h]     [128s, HPC*128v] per s-tile
            knT_sb = [
                res.tile([NOPE, S], BF16, tag=f"knt{h}", name=f"knt{h}")
                for h in range(HPC)
            ]
            v_sb = res.tile([128, NST, HPC * VDIM], BF16, tag="vsb")
            for sb in range(NSB):
                lo, hi = sb * 512, (sb + 1) * 512
                for h in range(HPC):
                    kn_ps = ps_sc.tile([128, 512], FP32, tag="sc", name="kn_ps")
                    for c in range(4):
                        nc.tensor.matmul(
                            kn_ps[:],
                            wkcT_sb[:, h, c, :],
                            kvT_sb[c][:, lo:hi],
                            kind="knope",
                            start=(c == 0),
                            stop=(c == 3),
                        )
                    nc.vector.tensor_copy(knT_sb[h][:, lo:hi], kn_ps[:])
                for k in range(4):
                    st = sb * 4 + k
                    v_ps = ps_sc.tile([128, HPC * VDIM], FP32, tag="sc",
                                      name="v_ps")
                    for c in range(4):
                        nc.tensor.matmul(
                            v_ps[:],
                            kvT_sb[c][:, st * 128:(st + 1) * 128],
                            wvc_sb[:, c, :],
                            kind="vproj",
                            start=(c == 0),
                            stop=(c == 3),
                        )
                    nc.scalar.copy(v_sb[:, st, :], v_ps[:])

            # ---------------- main phases: (head, t-block)
            # s-tiles are processed in PAIRS: scores for two s-tiles land in
            # one 2-bank PSUM tile so a single ACTIVATE exponentiates both
            # (halves the per-call ACT overhead). Denominator partials
            # accumulate into 4 rotating bf16 accumulators so the DVE adds
            # hit the 2x packed-bf16 mode; partial magnitudes stay ~16 tiles
            # so bf16 rounding stays negligible. The epilogue tail of phase
            # p (broadcast matmul, scale, DMA) is emitted a few pairs into
            # phase p+1 so the in-order PE queue never stalls on it.
            deferred = None
            for ph in range(HPC * NTB):
                h, tb = divmod(ph, NTB)
                tlo, thi = tb * 512, (tb + 1) * 512
                attn_ps = ps_attn.tile([128, 512], FP32, tag="attn")
                accs = [accp.tile([128, 512], BF16, tag="acc", name="acc")
                        for _ in range(4)]
                pending = []  # (s0, pt2) of previous pairs: PV runs two
                # pairs behind so its exp dependency is long satisfied

                def emit_pv(s0, pt2, attn=attn_ps, hh=h):
                    for i in range(2):
                        s = s0 + i
                        nc.tensor.matmul(
                            attn[:],
                            v_sb[:, s, hh * VDIM:(hh + 1) * VDIM],
                            pt2[:, i, :],
                            kind="pv",
                            start=(s == 0),
                            stop=(s == NST - 1),
                        )

                for pp in range(NST // 2):
                    s0 = 2 * pp
                    sc2 = ps_sc.tile([128, 2, 512], FP32, tag="sc",
                                     name="sc2_ps")
                    # the two K=64 rope matmuls run concurrently in disjoint
                    # PE row-groups (rope rows are duplicated to 64:128)
                    for i in range(2):
                        rlo = i * ROPE
                        nc.tensor.matmul(
                            sc2[:, i, :],
                            kvr_sb[rlo:rlo + ROPE,
                                   (s0 + i) * 128:(s0 + i + 1) * 128],
                            qpe_sb[h][rlo:rlo + ROPE, tlo:thi],
                            kind="rope",
                            start=True,
                            stop=False,
                            tile_position=(rlo, 0),
                        )
                    for i in range(2):
                        s = s0 + i
                        nc.tensor.matmul(
                            sc2[:, i, :],
                            knT_sb[h][:, s * 128:(s + 1) * 128],
                            qnT_sb[:, h * T + tlo:h * T + thi],
                            kind="nope",
                            start=False,
                            stop=True,
                        )
                    pt2 = ptp.tile([128, 2, 512], BF16, tag="pt", name="pt2")
                    nc.scalar.activation(pt2[:], sc2[:], Exp, scale=SCALING)
                    for i in range(2):
                        s = s0 + i
                        a = accs[s % 4]
                        if s < 4:
                            nc.vector.tensor_copy(a[:], pt2[:, i, :])
                        else:
                            nc.vector.tensor_add(a[:], a[:], pt2[:, i, :])

                    pending.append((s0, pt2))
                    if len(pending) > 2:
                        emit_pv(*pending.pop(0))
                    if pp == 4 and deferred is not None:
                        deferred()
                        deferred = None
                for p in pending:
                    emit_pv(*p)

                # phase epilogue head: total denominator via 4 accumulated
                # K=128 matmuls, then its reciprocal (approx, ~18 bits) on
                # the [1,512] row
                den_ps = ps_sc.tile([1, 512], FP32, tag="sc", name="den_ps")
                for j in range(4):
                    nc.tensor.matmul(den_ps[:], ones_col[:], accs[j][:],
                                     kind="den", start=(j == 0), stop=(j == 3))
                rcp = smsb.tile([1, 512], FP32, tag="rcp")
                nc.vector.reciprocal(rcp[:], den_ps[:])

                def deferred(attn=attn_ps, rc=rcp, hh=h, lo=tlo, hi=thi):
                    # epilogue tail: broadcast 1/denom across partitions via
                    # K=1 matmul, scale attnT, DMA out
                    bc_ps = ps_sc.tile([128, 512], FP32, tag="sc",
                                       name="bc_ps")
                    nc.tensor.matmul(bc_ps[:], ones_row[:], rc[:],
                                     kind="bcast")
                    bc_sb = smsb.tile([128, 512], FP32, tag="bc")
                    nc.scalar.copy(bc_sb[:], bc_ps[:])
                    ot = outsb.tile([128, 512], FP32, tag="out")
                    nc.vector.tensor_mul(ot[:], attn[:], bc_sb[:])
                    nc.sync.dma_start(
                        out[hh * VDIM:(hh + 1) * VDIM, lo:hi], ot[:]
                    )

            deferred()
    return nc


_PROGRAM = None


def _get_program():
    global _PROGRAM
    if _PROGRAM is None:
        _PROGRAM = _build_program()
    return _PROGRAM


# ---------------------------------------------------------------- host side
last_results = None  # BassKernelResults of the most recent run (for test.py)


def kernel(q, kv_cache, w_kc, w_vc):
    q = np.asarray(q, dtype=np.float32)
    kv_cache = np.asarray(kv_cache, dtype=np.float32)
    w_kc = np.asarray(w_kc, dtype=np.float32)
    w_vc = np.asarray(w_vc, dtype=np.float32)

    kvT_full = np.ascontiguousarray(kv_cache.T).astype(NPBF)       # [576, S]
    kvT_np = kvT_full[:LORA]                                        # [512, S]
    kvr_np = np.concatenate([kvT_full[LORA:], kvT_full[LORA:]], 0)  # [128, S] rope x2

    in_maps = []
    for core in range(N_CORES):
        hs = [core * HPC + i for i in range(HPC)]
        qnT_np = np.stack(
            [np.ascontiguousarray(q[:, h, :NOPE].T) for h in hs]
        ).astype(NPBF)                                              # [HPC,128,T]
        qpT_np = np.stack(
            [np.ascontiguousarray(q[:, h, NOPE:].T) for h in hs]
        ).astype(NPBF)                                              # [HPC,64,T]
        wkcT_np = np.stack(
            [np.ascontiguousarray(w_kc[h].T) for h in hs]
        ).astype(NPBF)                                              # [HPC,512,128]
        wvc2_np = np.stack(
            [
                np.concatenate(
                    [w_vc[h][c * 128:(c + 1) * 128, :] for h in hs], axis=1
                )
                for c in range(4)
            ]
        ).astype(NPBF)                                              # [4,128,HPC*128]
        in_maps.append(
            {
                "qnT": qnT_np,
                "qpT": qpT_np,
                "kvT": kvT_np,
                "kvr": kvr_np,
                "wkcT": wkcT_np,
                "wvc2": wvc2_np,
            }
        )

    nc = _get_program()
    trace = bool(int(os.environ.get("KERNEL_TRACE", "0")))
    trace_cores = None
    if trace and os.environ.get("KERNEL_TRACE_CORES"):
        trace_cores = [
            int(x) for x in os.environ["KERNEL_TRACE_CORES"].split(",")
        ]
    res = run_bass_kernel_spmd(
        nc,
        in_maps,
        core_ids=list(range(N_CORES)),
        trace=trace,
        trace_cores=trace_cores,
    )
    global last_results
    last_results = res

    # per-core out is [HPC*VDIM, T] transposed; concat heads then transpose
    full = np.concatenate([res.results[c]["out"] for c in range(N_CORES)], axis=0)
    return np.ascontiguousarray(full.T.astype(np.float32))


# revision 16
# speedup vs baseline: 1.2363x; 1.0369x over previous
"""DeepseekV2 MLA (non-absorbed prefill form, chunked-softmax MQA) on 8 trn2
NeuronCores.

Sharding: tensor-parallel over heads (16 heads / 8 cores = 2 heads per core);
the 576-wide latent KV cache is replicated per core. Each core computes its two
heads' attention output transposed [256, 1024]; the host concatenates along
heads and transposes back. All matmuls run in bf16 with fp32 PSUM accumulation.

With T=1024 queries (prefill), materializing per-head K/V from the latent
cache is far cheaper than the weight-absorbed decode form: the score
contraction drops 576->192 and PV drops 512->128, at the cost of two
S x 512 x 128 projections per head, amortized over all queries.

Per-core dataflow (transposed [d, t] layouts; no on-chip transposes):
  preprocessing, per s-block (PE + DVE):
    k_nopeT = w_kcT_chunk.T @ kvT_chunk   (PE, accum 4 l-chunks) [128n, 512s]
    v       = kvT_chunk.T @ w_vc_2heads   (PE, accum 4 l-chunks) [128s, 256v]
  main loop, per (head, t-block) phase, per s-tile (PE + ACT + DVE):
    scoresT = ropeT.T @ q_peT  (K=64, paired row groups)
            + k_nopeT.T @ q_nopeT         (PE)                   [128s, 512t]
    pT      = exp(scale * scoresT)        (ACT, PSUM->SBUF bf16)
    acc    += pT                          (DVE, denominator partials)
    attnT  += v_tile.T @ pT               (PE, accum 64 s-tiles) [128v, 512t]
  phase epilogue:
    denom   = ones.T @ acc                (PE, K=128)            [1, 512t]
    recip   = 1/denom                     (DVE)
    bcast   = ones_row.T @ recip          (PE, K=1)              [128, 512t]
    outT    = attnT * bcast               (DVE), DMA out [128v, 512t].
"""

import os
import sys

import numpy as np
import ml_dtypes

for _p in ("/opt/trn_rl_repo",):
    if os.path.isdir(_p) and _p not in sys.path:
        sys.path.append(_p)

import concourse.bass as bass
import concourse.mybir as mybir
import concourse.tile as tile
from concourse.bass_utils import run_bass_kernel_spmd
from concourse.vector_clock import ScopedClock, VectorClock

# ---------------------------------------------------------------- constants
NOPE, ROPE, LORA, VDIM = 128, 64, 512, 128
T, H, S = 1024, 16, 8192
D = LORA + ROPE            # 576 latent dim
SCALING = (NOPE + ROPE) ** -0.5
N_CORES = 8
HPC = H // N_CORES         # heads per core
NST = S // 128             # 64 s-tiles
NSB = S // 512             # 16 s-blocks
NTB = T // 512             # 2 t-blocks
BF16 = mybir.dt.bfloat16
FP32 = mybir.dt.float32
NPBF = ml_dtypes.bfloat16


# ------------------------------------------------- walrus drain workaround
def _patch_tile_drain():
    """The neuronxcc walrus in this container rejects DRAIN instructions
    carrying more than ~2 sync waits ("Too many sync wait commands").
    Split the TileContext exit drain into one drain per processor tick;
    the waits execute sequentially on SP before the all-engine barrier,
    preserving the original semantics."""
    if getattr(tile.TileContext, "_drain_split_patched", False):
        return

    def _drain_and_barrier_split(self, tick_clock, wait_clock):
        gcv = tick_clock.global_clock
        n = len(gcv)
        for proc in range(n):
            t = gcv[proc]
            if t <= 0:
                continue
            vc = VectorClock([0] * n)
            vc.require_at_least(proc, t)
            d = self.nc.sync.drain()
            wait_clock.add_sem_waits(d.ins, ScopedClock({None: vc}))
        self.nc.all_engine_barrier()
        assert self.sems is not None
        popped = self.nc._tile_sem_poison_stack.pop()
        assert popped is self._sem_poison
        self.nc.clear_and_free_semaphores(list(self.sems.allocated().values()))
        self.nc.all_engine_barrier()

    tile.TileContext._drain_and_barrier = _drain_and_barrier_split

    # Same walrus limitation for regular instructions: peel all but the last
    # sync wait off onto same-engine NOPs inserted immediately before the
    # instruction. The engine executes its queue in order, so waiting on the
    # NOPs first is equivalent to one multi-wait instruction.
    orig_add = tile.TileContext._add_instruction

    def _add_instruction_split_waits(self, inst):
        si = inst.sync_info
        if si is not None:
            waits = si.on_wait
            if waits and len(waits) > 1:
                for w in waits[:-1]:
                    nop = mybir.InstNoOp(
                        name=self.nc.get_next_instruction_name(), ins=[], outs=[]
                    )
                    nop.engine = inst.engine
                    nop.sync_info = mybir.SyncInfo(on_wait=[w], on_update=[])
                    orig_add(self, nop)
                inst.sync_info = mybir.SyncInfo(
                    on_wait=[waits[-1]], on_update=si.on_update
                )
        orig_add(self, inst)

    tile.TileContext._add_instruction = _add_instruction_split_waits
    tile.TileContext._drain_split_patched = True


# ------------------------------------------------------------ bass program
MM_KINDS = {}


def _build_program():
    _patch_tile_drain()
    nc = bass.Bass()
    _orig_mm = nc.tensor.matmul

    def _mm_logged(out, lhsT, rhs, kind="?", **kw):
        inst = _orig_mm(out, lhsT, rhs, **kw)
        MM_KINDS[inst.ins.name] = kind
        return inst

    nc.tensor.matmul = _mm_logged
    qnT = nc.declare_dram_parameter("qnT", [HPC, NOPE, T], BF16, isOutput=False)
    qpT = nc.declare_dram_parameter("qpT", [HPC, ROPE, T], BF16, isOutput=False)
    kvT = nc.declare_dram_parameter("kvT", [LORA, S], BF16, isOutput=False)
    kvr = nc.declare_dram_parameter("kvr", [128, S], BF16, isOutput=False)
    wkcT = nc.declare_dram_parameter("wkcT", [HPC, LORA, NOPE], BF16, isOutput=False)
    wvc2 = nc.declare_dram_parameter("wvc2", [4, 128, HPC * VDIM], BF16, isOutput=False)
    out = nc.declare_dram_parameter("out", [HPC * VDIM, T], FP32, isOutput=True)

    Exp = mybir.ActivationFunctionType.Exp
    Ln = mybir.ActivationFunctionType.Ln

    with tile.TileContext(nc) as tc:
        with (
            tc.tile_pool(name="res", bufs=1) as res,
            tc.tile_pool(name="ptp", bufs=6) as ptp,
            tc.tile_pool(name="accp", bufs=8) as accp,
            tc.tile_pool(name="smsb", bufs=4) as smsb,
            tc.tile_pool(name="outsb", bufs=2) as outsb,
            tc.tile_pool(name="ps_sc", bufs=3, space="PSUM") as ps_sc,
            tc.tile_pool(name="ps_attn", bufs=2, space="PSUM") as ps_attn,
        ):
            # ---------------- resident loads. wkcT + kv block 0 first: they
            # unblock the preprocessing matmuls; q is only needed ~60us in.
            wkcT_sb = res.tile([128, HPC, 4, NOPE], BF16, tag="wkct")
            for h in range(HPC):
                for c in range(4):
                    nc.sync.dma_start(
                        wkcT_sb[:, h, c, :], wkcT[h, c * 128:(c + 1) * 128, :]
                    )

            kvT_sb = [
                res.tile([128, S], BF16, tag=f"kvt{c}", name=f"kvt{c}")
                for c in range(4)
            ]
            kvr_sb = res.tile([128, S], BF16, tag="kvr")

            def load_kv_block(b):
                nc.sync.dma_start(
                    kvr_sb[:, b * 1024:(b + 1) * 1024],
                    kvr[:, b * 1024:(b + 1) * 1024],
                )
                for c in range(4):
                    nc.sync.dma_start(
                        kvT_sb[c][:, b * 1024:(b + 1) * 1024],
                        kvT[c * 128:(c + 1) * 128, b * 1024:(b + 1) * 1024],
                    )

            load_kv_block(0)
            wvc_sb = res.tile([128, 4, HPC * VDIM], BF16, tag="wvc")
            for c in range(4):
                nc.sync.dma_start(wvc_sb[:, c, :], wvc2[c])
            for b in range(1, 4):
                load_kv_block(b)
            qnT_sb = res.tile([NOPE, HPC * T], BF16, tag="qnt")
            for h in range(HPC):
                nc.sync.dma_start(qnT_sb[:, h * T:(h + 1) * T], qnT[h])
            # rope rows are duplicated to partitions 64:128 so pairs of K=64
            # rope matmuls can run concurrently in disjoint PE row-groups
            qpe_sb = []
            for h in range(HPC):
                qp = res.tile([128, T], BF16, tag=f"qpe{h}")
                nc.scalar.dma_start(qp[0:ROPE, :], qpT[h])
                nc.scalar.dma_start(qp[ROPE:128, :], qpT[h])
                qpe_sb.append(qp)
            for b in range(4, 8):
                load_kv_block(b)
            ones_col = res.tile([128, 1], BF16, tag="ones_col")
            nc.vector.memset(ones_col[:], 1.0)
            ones_row = res.tile([1, 128], FP32, tag="ones_row")
            nc.vector.memset(ones_row[:], 1.0)

            # PE warmup: ~6us of matmuls on local data so HAM un-throttles and
            # the first real matmuls run at 2.4GHz instead of 1.2; also covers
            # the initial DMA latency of wkcT + kv block 0
            warm = res.tile([128, 512], BF16, tag="warm")
            nc.vector.memset(warm[:], 0.0)
            wu_ps = ps_sc.tile([128, 512], FP32, tag="sc", name="wu_ps")
            for _w in range(14):
                nc.tensor.matmul(wu_ps[:], warm[:, 0:128], warm[:, 0:512],
                                 kind="warm")

            # ---------------- preprocessing: per-head K_nope^T and V
            # k_nopeT[h] = w_kc[h] @ kv_lora^T   [128n, S]
            # v[s, :] = kv_lora[s] @ w_vc[h]     [128s, HPC*128v] per s-tile
            knT_sb = [
                res.tile([NOPE, S], BF16, tag=f"knt{h}", name=f"knt{h}")
                for h in range(HPC)
            ]
            v_sb = res.tile([128, NST, HPC * VDIM], BF16, tag="vsb")
            for sb in range(NSB):
                lo, hi = sb * 512, (sb + 1) * 512
                for h in range(HPC):
                    kn_ps = ps_sc.tile([128, 512], FP32, tag="sc", name="kn_ps")
                    for c in range(4):
                        nc.tensor.matmul(
                            kn_ps[:],
                            wkcT_sb[:, h, c, :],
                            kvT_sb[c][:, lo:hi],
                            kind="knope",
                            start=(c == 0),
                            stop=(c == 3),
                        )
                    nc.vector.tensor_copy(knT_sb[h][:, lo:hi], kn_ps[:])
                for k in range(4):
                    st = sb * 4 + k
                    v_ps = ps_sc.tile([128, HPC * VDIM], FP32, tag="sc",
                                      name="v_ps")
                    for c in range(4):
                        nc.tensor.matmul(
                            v_ps[:],
                            kvT_sb[c][:, st * 128:(st + 1) * 128],
                            wvc_sb[:, c, :],
                            kind="vproj",
                            start=(c == 0),
                            stop=(c == 3),
                        )
                    nc.scalar.copy(v_sb[:, st, :], v_ps[:])

            # ---------------- main phases: (head, t-block)
            # s-tiles are processed in PAIRS: scores for two s-tiles land in
            # one 2-bank PSUM tile so a single ACTIVATE exponentiates both
            # (halves the per-call ACT overhead). Denominator partials
            # accumulate into 4 rotating bf16 accumulators so the DVE adds
            # hit the 2x packed-bf16 mode; partial magnitudes stay ~16 tiles
            # so bf16 rounding stays negligible. The epilogue tail of phase
            # p (broadcast matmul, scale, DMA) is emitted a few pairs into
            # phase p+1 so the in-order PE queue never stalls on it.
            deferred_head = None
            deferred_tail = None
            for ph in range(HPC * NTB):
                h, tb = divmod(ph, NTB)
                tlo, thi = tb * 512, (tb + 1) * 512
                attn_ps = ps_attn.tile([128, 512], FP32, tag="attn")
                accs = [accp.tile([128, 512], BF16, tag="acc", name="acc")
                        for _ in range(4)]
                pending = []  # (s0, pt2) of previous pairs: PV runs two
                # pairs behind so its exp dependency is long satisfied

                def emit_pv(s0, pt2, attn=attn_ps, hh=h):
                    for i in range(2):
                        s = s0 + i
                        nc.tensor.matmul(
                            attn[:],
                            v_sb[:, s, hh * VDIM:(hh + 1) * VDIM],
                            pt2[:, i, :],
                            kind="pv",
                            start=(s == 0),
                            stop=(s == NST - 1),
                        )

                for pp in range(NST // 2):
                    s0 = 2 * pp
                    sc2 = ps_sc.tile([128, 2, 512], FP32, tag="sc",
                                     name="sc2_ps")
                    # the two K=64 rope matmuls run concurrently in disjoint
                    # PE row-groups (rope rows are duplicated to 64:128)
                    for i in range(2):
                        rlo = i * ROPE
                        nc.tensor.matmul(
                            sc2[:, i, :],
                            kvr_sb[rlo:rlo + ROPE,
                                   (s0 + i) * 128:(s0 + i + 1) * 128],
                            qpe_sb[h][rlo:rlo + ROPE, tlo:thi],
                            kind="rope",
                            start=True,
                            stop=False,
                            tile_position=(rlo, 0),
                        )
                    for i in range(2):
                        s = s0 + i
                        nc.tensor.matmul(
                            sc2[:, i, :],
                            knT_sb[h][:, s * 128:(s + 1) * 128],
                            qnT_sb[:, h * T + tlo:h * T + thi],
                            kind="nope",
                            start=False,
                            stop=True,
                        )
                    pt2 = ptp.tile([128, 2, 512], BF16, tag="pt", name="pt2")
                    nc.scalar.activation(pt2[:], sc2[:], Exp, scale=SCALING)
                    for i in range(2):
                        s = s0 + i
                        a = accs[s % 4]
                        if s < 4:
                            nc.vector.tensor_copy(a[:], pt2[:, i, :])
                        else:
                            nc.vector.tensor_add(a[:], a[:], pt2[:, i, :])

                    pending.append((s0, pt2))
                    if len(pending) > 2:
                        emit_pv(*pending.pop(0))
                    if pp == 1 and deferred_head is not None:
                        deferred_tail = deferred_head()
                        deferred_head = None
                    if pp == 5 and deferred_tail is not None:
                        deferred_tail()
                        deferred_tail = None
                for p in pending:
                    emit_pv(*p)

                # The phase epilogue is deferred into the next phase so the
                # in-order PE queue never waits on the DVE add tail or the
                # reciprocal: head (denominator matmuls + 1/denom via
                # exp(-ln(d)) on ACT) lands after pair 1, tail (broadcast
                # matmul, scale, DMA) after pair 5.
                def _make_head(attn=attn_ps, acs=accs, hh=h, lo=tlo, hi=thi):
                    def head():
                        den_ps = ps_sc.tile([1, 512], FP32, tag="sc",
                                            name="den_ps")
                        for j in range(4):
                            nc.tensor.matmul(den_ps[:], ones_col[:],
                                             acs[j][:], kind="den",
                                             start=(j == 0), stop=(j == 3))
                        lnd = smsb.tile([1, 512], FP32, tag="lnd")
                        nc.scalar.activation(lnd[:], den_ps[:], Ln)
                        rcp = smsb.tile([1, 512], FP32, tag="rcp")
                        nc.scalar.activation(rcp[:], lnd[:], Exp, scale=-1.0)

                        def tail():
                            bc_ps = ps_sc.tile([128, 512], FP32, tag="sc",
                                               name="bc_ps")
                            nc.tensor.matmul(bc_ps[:], ones_row[:], rcp[:],
                                             kind="bcast")
                            bc_sb = smsb.tile([128, 512], FP32, tag="bc")
                            nc.scalar.copy(bc_sb[:], bc_ps[:])
                            ot = outsb.tile([128, 512], FP32, tag="out")
                            nc.vector.tensor_mul(ot[:], attn[:], bc_sb[:])
                            nc.sync.dma_start(
                                out[hh * VDIM:(hh + 1) * VDIM, lo:hi], ot[:]
                            )

                        return tail

                    return head

                deferred_head = _make_head()

            # final phase: run its epilogue immediately
            deferred_head()()
    return nc


_PROGRAM = None


def _get_program():
    global _PROGRAM
    if _PROGRAM is None:
        _PROGRAM = _build_program()
    return _PROGRAM


# ---------------------------------------------------------------- host side
last_results = None  # BassKernelResults of the most recent run (for test.py)


def kernel(q, kv_cache, w_kc, w_vc):
    q = np.asarray(q, dtype=np.float32)
    kv_cache = np.asarray(kv_cache, dtype=np.float32)
    w_kc = np.asarray(w_kc, dtype=np.float32)
    w_vc = np.asarray(w_vc, dtype=np.float32)

    kvT_full = np.ascontiguousarray(kv_cache.T).astype(NPBF)       # [576, S]
    kvT_np = kvT_full[:LORA]                                        # [512, S]
    kvr_np = np.concatenate([kvT_full[LORA:], kvT_full[LORA:]], 0)  # [128, S] rope x2

    in_maps = []
    for core in range(N_CORES):
        hs = [core * HPC + i for i in range(HPC)]
        qnT_np = np.stack(
            [np.ascontiguousarray(q[:, h, :NOPE].T) for h in hs]
        ).astype(NPBF)                                              # [HPC,128,T]
        qpT_np = np.stack(
            [np.ascontiguousarray(q[:, h, NOPE:].T) for h in hs]
        ).astype(NPBF)                                              # [HPC,64,T]
        wkcT_np = np.stack(
            [np.ascontiguousarray(w_kc[h].T) for h in hs]
        ).astype(NPBF)                                              # [HPC,512,128]
        wvc2_np = np.stack(
            [
                np.concatenate(
                    [w_vc[h][c * 128:(c + 1) * 128, :] for h in hs], axis=1
                )
                for c in range(4)
            ]
        ).astype(NPBF)                                              # [4,128,HPC*128]
        in_maps.append(
            {
                "qnT": qnT_np,
                "qpT": qpT_np,
                "kvT": kvT_np,
                "kvr": kvr_np,
                "wkcT": wkcT_np,
                "wvc2": wvc2_np,
            }
        )

    nc = _get_program()
    trace = bool(int(os.environ.get("KERNEL_TRACE", "0")))
    trace_cores = None
    if trace and os.environ.get("KERNEL_TRACE_CORES"):
        trace_cores = [
            int(x) for x in os.environ["KERNEL_TRACE_CORES"].split(",")
        ]
    res = run_bass_kernel_spmd(
        nc,
        in_maps,
        core_ids=list(range(N_CORES)),
        trace=trace,
        trace_cores=trace_cores,
    )
    global last_results
    last_results = res

    # per-core out is [HPC*VDIM, T] transposed; concat heads then transpose
    full = np.concatenate([res.results[c]["out"] for c in range(N_CORES)], axis=0)
    return np.ascontiguousarray(full.T.astype(np.float32))
